# revision 20
# baseline (speedup 1.0000x reference)
"""KAN 3x3 convolution kernel for 8 Trainium2 NeuronCores.

Math: out[b,o,ih,iw] = sum_{c,k} scale_base[o,c,k]*silu(t) + sum_{c,k,m} W_sp[o,c,k,m]*B3_m(t)
where t = xpad[b,c,ih+di,iw+dj] for kernel position k=(di,dj), and B3_m is the
cubic B-spline basis on the uniform extended grid [-2.2, 2.2], h=0.4.

Reflection identity with constants folded so the combine is a plain subtract:
  B3_m(t) = (P^3 - Q^3)/12,  z = |2.5t + 3.5 - m|
  P = relu(2c - c*z) (c = 2^{1/3}),  Q = relu(2 - 2z)
All features bounded (P<=2.52, Q<=2) so bf16 rounding is benign; matmuls in
bf16 (1 col/cycle on the PE, 2x DVE elementwise).

Per core: one batch element. 21 weight groups x 2 column blocks of 512 pixels:
  groups 0-2:  silu chunk, 3 vertical shifts baked into partitions (97 rows
               incl. const-1 bias row) -> only the horizontal shift dj remains
               in the rhs view (3 matmuls instead of 9)
  groups 3-20: two spline chunks (4 m-values x 32 channels = 128 rows) x 9
               kernel positions via shifted rhs views
Weight matrices ping-pong between PE column tiles (0,0)/(0,64) so loads hide
under streaming; each group's second matmul reuses the loaded weights
(ldweights=False). psum halves are summed in the epilogue.
"""
import sys
sys.path.insert(0, '/opt/trn_rl_repo')
import warnings
warnings.filterwarnings('ignore')
import numpy as np
import ml_dtypes

import concourse.bass as bass
import concourse.mybir as mybir
import concourse.tile as tile
from concourse.bass_utils import run_bass_kernel_spmd

B, C, O, H, W = 8, 32, 64, 32, 32
KH = KW = 3
NUM, KS = 5, 3
M = NUM + KS            # 8 spline bases
HP = WP = 34            # padded grid
NPIX = HP * WP          # 1156
NG = 21                 # weight groups
DT = mybir.dt.bfloat16
CBRT2 = 2.0 ** (1.0 / 3.0)

USE_TILEPOS = True      # ping-pong weight tiles (0,0)/(0,64)
USE_LDW_SKIP = True     # reuse loaded weights for the 2nd column block


class _TC(tile.TileContext):
    """TileContext whose final drain splits sem waits to <=2 per instruction
    (walrus CTRL codegen rejects drains with too many sync waits)."""

    def _drain_and_barrier(self, tick_clock, wait_clock):
        from concourse.vector_clock import ScopedClock
        nc = self.nc
        drain_inst = nc.sync.drain()
        wait_clock.add_sem_waits(
            drain_inst.ins, ScopedClock({None: tick_clock.global_clock})
        )
        si = drain_inst.ins.sync_info
        waits = list(si.on_wait or [])
        MAXW = 1
        if len(waits) > MAXW:
            del si.on_wait[MAXW:]
            rest = waits[MAXW:]
            for i in range(0, len(rest), MAXW):
                d2 = nc.sync.drain()
                s2 = d2.ins.sync_info
                if s2 is None:
                    s2 = type(si)(on_wait=[], on_update=[])
                    d2.ins.sync_info = s2
                s2.on_wait.extend(rest[i:i + MAXW])
        nc.all_engine_barrier()
        popped = nc._tile_sem_poison_stack.pop()
        assert popped is self._sem_poison
        nc.clear_and_free_semaphores(list(self.sems.allocated().values()))
        nc.all_engine_barrier()


# matmul issue order: cb0 (groups 3-11), silu (0-2), cb1 (12-20); weights are
# laid out in DRAM in this order so the first-used groups arrive first.
GORDER = list(range(3, 12)) + [0, 1, 2] + list(range(12, NG))
NG1 = 8                 # groups in the first weights DMA


def _host_weights(coef, scale_base, scale_sp, bias):
    """bf16 lhsT weights [128, 21, 64] in GORDER: silu groups (rows 32*di+c,
    const/bias row 96 in group dj=1), spline groups 3+9*jc+k (rows 32*ml+c
    hold W_sp[o,c,k,4jc+ml]/12)."""
    W_sp = (scale_sp[..., None] * coef).astype(np.float32)   # (O, C, 9, M)
    wfull = np.zeros((128, NG, O), dtype=np.float32)
    for dj in range(3):
        for di in range(3):
            wfull[32 * di:32 * di + 32, dj, :] = (
                scale_base[:, :, 3 * di + dj].T)             # (c, o)
    wfull[96, 1, :] = bias
    for jc in range(2):
        for k in range(KH * KW):
            for ml in range(4):
                wfull[32 * ml:32 * ml + 32, 3 + 9 * jc + k, :] = (
                    W_sp[:, :, k, 4 * jc + ml].T / 12.0)
    return wfull[:, GORDER, :].astype(ml_dtypes.bfloat16)


def _build_nc():
    nc = bass.Bass()
    x_d = nc.dram_tensor("xpad", [32, NPIX], DT, kind="ExternalInput")
    w_d = nc.dram_tensor("wfull", [128, NG * O], DT, kind="ExternalInput")
    o_d = nc.dram_tensor("out", [O, H * W], mybir.dt.float32, kind="ExternalOutput")

    # Per-partition scalar columns (z biases per m-block, P/Q biases),
    # memset before the TileContext like the framework's own const APs.
    bc_t = nc.alloc_sbuf_tensor("bcols", [128, 4], DT)
    bc = bc_t.ap()
    for jc in range(2):
        for ml in range(4):
            nc.gpsimd.memset(bc[32 * ml:32 * ml + 32, jc:jc + 1],
                             3.5 - (4 * jc + ml))
    nc.gpsimd.memset(bc[:, 2:3], 2.0 * CBRT2)
    nc.gpsimd.memset(bc[:, 3:4], 2.0)
    nc.all_engine_barrier()

    with _TC(nc) as tc:
        with tc.tile_pool(name="sb", bufs=1) as sb, \
             tc.tile_pool(name="eps", bufs=2) as eps, \
             tc.tile_pool(name="ps", bufs=1, space="PSUM") as ps:
            xb = sb.tile([128, NPIX], DT)
            # x DMA triggered from the (otherwise idle-at-start) ACT hwdge
            # queue; the DRAM side is read once and partition-broadcast x4.
            nc.scalar.dma_start(out=xb[:, :],
                                in_=x_d[:, :].partition_broadcast(4))
            xq = xb[:, 0:NPIX].rearrange("p (h w) -> p h w", h=HP)

            wf = sb.tile([128, NG * O], DT)
            # weights in two use-ordered chunks on the gpsimd swdge ring
            nc.gpsimd.dma_start(out=wf[:, 0:NG1 * O], in_=w_d[:, 0:NG1 * O])
            nc.gpsimd.dma_start(out=wf[:, NG1 * O:], in_=w_d[:, NG1 * O:])

            AF = mybir.ActivationFunctionType
            # ---- tiles. All matmul-facing writes go through DVE (silu staged
            # via sl) so matmuls carry one sem wait (walrus rejects >1 on MM).
            F = sb.tile([97, HP, WP], DT)
            sl = sb.tile([32, HP, WP], DT)
            zt, Pt, Qt, P2t, P3t, Q2t, Q3t, chunks = [], [], [], [], [], [], [], []
            for jc in range(2):
                z = sb.tile([128, HP, WP], DT, tag=f"z{jc}")
                P = sb.tile([128, HP, WP], DT, tag=f"P{jc}")
                Q = sb.tile([128, HP, WP], DT, tag=f"Q{jc}")
                P2 = sb.tile([128, HP, WP], DT, tag=f"P2{jc}")
                P3 = sb.tile([128, HP, WP], DT, tag=f"P3{jc}")
                Q2 = sb.tile([128, HP, WP], DT, tag=f"Q2{jc}")
                Q3 = sb.tile([128, HP, WP], DT, tag=f"Q3{jc}")
                cb = sb.tile([128, HP, WP], DT, tag=f"cb{jc}")
                zt.append(z); Pt.append(P); Qt.append(Q); P2t.append(P2)
                P3t.append(P3); Q2t.append(Q2); Q3t.append(Q3); chunks.append(cb)

            nc.vector.memset(F[96:97, :, :], 1.0)

            # ---- features in two h-halves so lc=0 matmuls start early ----
            # half 0 covers grid rows [0,18) (lc=0 views), half 1 [18,34).
            for hh, (a, b) in enumerate(((0, 18), (18, 34))):
                sa, sb_ = (0, 20) if hh == 0 else (20, HP)
                for jc in range(2):
                    z, P, Q = zt[jc], Pt[jc], Qt[jc]
                    P2, P3, Q2, Q3, cb = (P2t[jc], P3t[jc], Q2t[jc], Q3t[jc],
                                          chunks[jc])
                    nc.scalar.activation(z[:, a:b, :], xq[:, a:b, :], AF.Abs,
                                         bias=bc[:, jc:jc + 1],
                                         scale=2.5)
                    nc.scalar.activation(P[:, a:b, :], z[:, a:b, :], AF.Relu,
                                         bias=bc[:, 2:3],
                                         scale=-CBRT2)
                    nc.scalar.activation(Q[:, a:b, :], z[:, a:b, :], AF.Relu,
                                         bias=bc[:, 3:4],
                                         scale=-2.0)
                    nc.vector.tensor_mul(P2[:, a:b, :], P[:, a:b, :], P[:, a:b, :])
                    nc.vector.tensor_mul(P3[:, a:b, :], P2[:, a:b, :], P[:, a:b, :])
                    nc.vector.tensor_mul(Q2[:, a:b, :], Q[:, a:b, :], Q[:, a:b, :])
                    nc.vector.tensor_mul(Q3[:, a:b, :], Q2[:, a:b, :], Q[:, a:b, :])
                    nc.vector.tensor_sub(cb[:, a:b, :], P3[:, a:b, :], Q3[:, a:b, :])
                    if jc == 0:
                        # silu half for F: rows [sa, sb_) of the padded grid
                        nc.scalar.activation(sl[:, sa:sb_, :], xq[0:32, sa:sb_, :],
                                             AF.Silu)
                        # F rows are only read for grid rows < 32 (view
                        # windows [0,16) and [16,32)): clamp the copies.
                        cl = min(b, 32)
                        nc.vector.tensor_copy(F[0:32, a:cl, :], sl[:, a:cl, :])
                        nc.vector.tensor_copy(F[32:64, a:cl, :],
                                              sl[:, a + 1:cl + 1, :])
                        nc.vector.tensor_copy(F[64:96, a:cl, :],
                                              sl[:, a + 2:cl + 2, :])

            # ---- 42 accumulating matmuls + epilogue ----
            psum = []
            for lc in range(2):
                pst = ps.tile([128, 512], mybir.dt.float32, tag=f"ps{lc}")
                psum.append(pst)
            # Dummy 1x1 matmuls into a scratch psum bank absorb the weight
            # DMA sem waits on the PE queue, so real matmuls carry only their
            # rhs (DVE) wait (walrus allows <=1 sem wait per matmul).
            psd = ps.tile([1, 1], mybir.dt.float32, tag="psd")
            nc.tensor.matmul(psd[0:1, 0:1], wf[0:1, 0:1], wf[0:1, 0:1],
                             start=True, stop=True)
            # lc outer: all 21 groups for block 0 (h-half 0 features), then
            # block 1. Group order cb0, silu, cb1 matches feature readiness.
            for lc in range(2):
                for gi, g in enumerate(GORDER):
                    if lc == 0 and gi == NG1:
                        nc.tensor.matmul(
                            psd[0:1, 0:1], wf[0:1, NG1 * O:NG1 * O + 1],
                            wf[0:1, NG1 * O:NG1 * O + 1], start=True, stop=True)
                    pos = (gi % 2) * 64 if USE_TILEPOS else 0
                    if g < 3:
                        rows, dj = 97, g
                        rhs = F[0:rows, lc * 16: lc * 16 + 16, dj: dj + 32]
                    else:
                        jc, k = (g - 3) // 9, (g - 3) % 9
                        rows, ik, jk = 128, k // 3, k % 3
                        rhs = chunks[jc][0:rows,
                                         lc * 16 + ik: lc * 16 + ik + 16,
                                         jk: jk + 32]
                    start = gi < (2 if USE_TILEPOS else 1)
                    stop = gi >= NG - (2 if USE_TILEPOS else 1)
                    nc.tensor.matmul(
                        psum[lc][pos: pos + O, :],
                        wf[0:rows, O * gi: O * gi + O],
                        rhs, start=start, stop=stop,
                        tile_position=(0, pos) if USE_TILEPOS else None)
            for lc in range(2):
                ot = eps.tile([O, 512], mybir.dt.float32, tag="ot")
                if USE_TILEPOS:
                    # DVE-only epilogue (one PSUM operand per op): copy the lo
                    # half (finishes last, so its PE wait covers the hi half),
                    # then the add carries no waits.
                    oh = eps.tile([O, 512], mybir.dt.float32, tag="oh")
                    nc.vector.tensor_copy(oh[:, :], psum[lc][0:O, :])
                    nc.vector.tensor_add(ot[:, :], psum[lc][O:2 * O, :], oh[:, :])
                else:
                    nc.vector.tensor_copy(ot[:, :], psum[lc][0:O, :])
                nc.sync.dma_start(out=o_d[:, 512 * lc: 512 * (lc + 1)],
                                  in_=ot[:, :])
    return nc


_NC_CACHE = {}


def _get_nc():
    if "nc" not in _NC_CACHE:
        _NC_CACHE["nc"] = _build_nc()
    return _NC_CACHE["nc"]


def _run(x, coef, scale_base, scale_sp, bias, trace=False):
    nc = _get_nc()
    kw = {}
    if trace:
        import os
        td = os.environ.get("KAN_TRACE_DIR")
        if td:
            os.makedirs(td, exist_ok=True)
            kw["tmpdir"] = td
    wfull = _host_weights(np.asarray(coef), np.asarray(scale_base),
                          np.asarray(scale_sp), np.asarray(bias))
    wflat = np.ascontiguousarray(wfull.reshape(128, -1))
    x = np.asarray(x)
    in_maps = []
    for b in range(B):
        xpad = np.zeros((C, HP, WP), dtype=np.float32)
        xpad[:, 1:1 + H, 1:1 + W] = x[b]
        xb = xpad.reshape(C, NPIX).astype(ml_dtypes.bfloat16)
        in_maps.append({"xpad": np.ascontiguousarray(xb), "wfull": wflat})
    res = run_bass_kernel_spmd(nc, in_maps, core_ids=list(range(B)), trace=trace,
                               **kw)
    out = np.stack([res.results[b]["out"].reshape(O, H, W) for b in range(B)])
    return out, res


def kernel(x, coef, scale_base, scale_sp, bias):
    out, _ = _run(x, coef, scale_base, scale_sp, bias, trace=False)
    return out


def kernel_traced(x, coef, scale_base, scale_sp, bias):
    # test.py injects the NTFF hook (antenv.axon_hooks) before importing us;
    # if absent, bass_utils degrades to untraced gracefully.
    out, res = _run(x, coef, scale_base, scale_sp, bias, trace=True)
    return out, res
